# revision 1
# baseline (speedup 1.0000x reference)
"""2-layer LSTM (B=128, T=256, D=512, H=1024) + linear head + ELU on 8 trn2 cores.

Strategy (all hardcoded; v6):
  - Feature-major compute: gates^T [1024, B], h^T [H, B], c^T [H_local, B].
    Full batch B=128 as the matmul moving dim, weights as the 128x128
    stationary operand -> full PE utilization, no transposes anywhere.
  - Sharding: dies fully redundant; 4-way tensor-parallel over the hidden dim
    within a die. Core s owns hidden slice [256s, 256s+256) = 1024 local gate
    rows, ordered [i_lo f_lo g_lo o_lo | i_hi f_hi g_hi o_hi] x 128 so each
    PSUM bank holds an i/f/g/o quartet.
  - Communication (the bottleneck - remote-DMA descriptors are processed with
    ~0.5us serial latency per descriptor per lane, so descriptor COUNT rules):
    ONE combined send group per step carrying [h0_t | h1_{t-1}] slices
    together (1KB per partition per dest = half the descriptors of separate
    sends), to the 3 XOR die peers only - the self slice is written in place
    by the elementwise (slot 0 of the gather IS the send source, no loopback
    traffic). Gather buffers are mod-3 rings; layout per slot k:
    [h0_lo h0_hi h1_lo h1_hi] of core (self^k), weight rows pre-permuted on
    host to match. A single semaphore (6 increments per step group) gates
    each step.
  - X^T streams in packed 4 steps per DMA ([128, 2KB] contiguous ->  4x fewer,
    4x bigger descriptors than per-step loads). xg0/xg1 are fused into the
    per-step accumulation (x-chunks first, h-chunks after the gather gate).
    Layer 1 lags layer 0 by one step.
  - Gate biases ride the ACT sigmoid/tanh ops as per-partition bias vectors.
    All weights SBUF-resident (bf16, ~60KB/partition); c state fp32.
"""

import sys
from contextlib import ExitStack

import ml_dtypes
import numpy as np

for _p in ("/opt/trn_rl_repo", "/root/.axon_site/_ro/trn_rl_repo"):
    if _p not in sys.path:
        sys.path.append(_p)

import concourse.bacc as bacc
import concourse.mybir as mybir
import concourse.tile as tile
from concourse.bass_utils import run_bass_kernel_spmd
from concourse.tile_rust import add_dep_helper

F32 = mybir.dt.float32
BF16 = mybir.dt.bfloat16
AF = mybir.ActivationFunctionType

P = 128
T = 256
D = 512
H = 1024
BR = 256
B = 128    # full batch on every core (dies are redundant)
HL = 256   # hidden units per core
NL = 1024  # local gate rows per core
NUM_CORES = 8


def _build(nc, n_steps):
    assert n_steps % 4 == 0
    xt_in = nc.dram_tensor("XT", [n_steps // 4, P, 2048], BF16,
                           kind="ExternalInput").ap()
    wx0_in = nc.dram_tensor("Wx0", [D, NL], BF16, kind="ExternalInput").ap()
    wh0_in = nc.dram_tensor("Wh0", [H, NL], BF16, kind="ExternalInput").ap()
    wx1_in = nc.dram_tensor("Wx1", [H, NL], BF16, kind="ExternalInput").ap()
    wh1_in = nc.dram_tensor("Wh1", [H, NL], BF16, kind="ExternalInput").ap()
    wbr_in = nc.dram_tensor("Wbr", [H, BR], BF16, kind="ExternalInput").ap()
    b0_in = nc.dram_tensor("b0p", [P, 8], F32, kind="ExternalInput").ap()
    b1_in = nc.dram_tensor("b1p", [P, 8], F32, kind="ExternalInput").ap()
    bbr_in = nc.dram_tensor("bbrp", [P, 2], F32, kind="ExternalInput").ap()
    y_out = nc.dram_tensor("y", [2, P, B], F32, kind="ExternalOutput").ap()

    sWx0 = nc.alloc_sbuf_tensor("sWx0", [P, 4, NL], BF16).ap()
    sWh0 = nc.alloc_sbuf_tensor("sWh0", [P, 8, NL], BF16).ap()
    sWx1 = nc.alloc_sbuf_tensor("sWx1", [P, 8, NL], BF16).ap()
    sWh1 = nc.alloc_sbuf_tensor("sWh1", [P, 8, NL], BF16).ap()
    sWbr = nc.alloc_sbuf_tensor("sWbr", [P, 8, BR], BF16).ap()
    sB0 = nc.alloc_sbuf_tensor("sB0", [P, 8], F32).ap()
    sB1 = nc.alloc_sbuf_tensor("sB1", [P, 8], F32).ap()
    sBbr = nc.alloc_sbuf_tensor("sBbr", [P, 2], F32).ap()

    # combined gather ring (stable address for remote writes): slot k holds
    # [h0_lo h0_hi h1_lo h1_hi] of core (self^k); slot 0 (self) doubles as
    # the send source.
    gath = nc.alloc_sbuf_tensor("gath", [P, 3, 16, B], BF16).ap()
    cst = [nc.alloc_sbuf_tensor(f"c{l}", [P, 2, B], F32).ap() for l in range(2)]

    rsems = [nc.alloc_semaphore(f"rsem{k}") for k in range(3)]
    lsem = nc.alloc_semaphore("lsem")

    patches = []

    def h0c(g, j):   # h0 chunk j view of a gather slot-major tile [P, 16, B]
        return g[:, 4 * (j // 2) + (j % 2)]

    def h1c(g, j):
        return g[:, 4 * (j // 2) + 2 + (j % 2)]

    with tile.TileContext(nc) as tc:
        barrier_nop = nc.gpsimd.nop(nofuse=True)

        for sb, src, nk in ((sWx0, wx0_in, 4), (sWh0, wh0_in, 8),
                            (sWx1, wx1_in, 8), (sWh1, wh1_in, 8)):
            v = src.rearrange("(k p) n -> k p n", p=P)
            for k in range(nk):
                nc.sync.dma_start(out=sb[:, k], in_=v[k])
        wbrv = wbr_in.rearrange("(k p) n -> k p n", p=P)
        for k in range(8):
            nc.sync.dma_start(out=sWbr[:, k], in_=wbrv[k])
        nc.sync.dma_start(out=sB0, in_=b0_in)
        nc.sync.dma_start(out=sB1, in_=b1_in)
        nc.sync.dma_start(out=sBbr, in_=bbr_in)
        nc.vector.memset(cst[0], 0.0)
        nc.vector.memset(cst[1], 0.0)

        stack = ExitStack()
        ps_pool = stack.enter_context(tc.tile_pool(name="psum", bufs=6, space="PSUM"))
        xt_pool = stack.enter_context(tc.tile_pool(name="xtp", bufs=3))
        tmp_pool = stack.enter_context(tc.tile_pool(name="tmp", bufs=8))
        hd_pool = stack.enter_context(tc.tile_pool(name="hdp", bufs=2))
        first_prep = [None]

        def gated_mms(mms_args, wait, chain_to=None):
            """Emit matmuls; the first carries `wait` (runtime patch) unless
            chain_to is given, in which case everything (including the first)
            is order-chained to that instruction instead."""
            first = chain_to
            for out, lhsT, rhs, start, stop in mms_args:
                mm = nc.tensor.matmul(out, lhsT, rhs, start=start, stop=stop)
                if first is None:
                    first = mm
                    if wait is not None:
                        patches.append((mm, wait[0], wait[1]))
                else:
                    add_dep_helper(mm.ins, first.ins, sync=False,
                                   reason="mms chained after gated first")
            return first

        def layer_mms(wx, nkx, xin_tiles, x_wait, wh, h_tiles, h_wait,
                      skip_rec):
            """One layer-step's matmuls into 2 psum quartets. Exactly one
            start=True per psum bank tile (a start clears has_written for the
            whole 2KiB bank row; later regions rely on per-element overwrite
            semantics). If x_wait is set, the first x-matmul carries it and
            everything else chains to it; otherwise h_wait goes on the first
            h-matmul (x-matmuls run ungated - they read DMA-tracked inputs)."""
            pss = [ps_pool.tile([P, 512], F32, name="ps") for _ in range(2)]

            def out_ap(j):
                return pss[j // 4][:, 128 * (j % 4) : 128 * (j % 4 + 1)]

            xargs = []
            for j in range(8):
                for kx in range(nkx):
                    xargs.append((out_ap(j), wx[:, kx, 128 * j : 128 * (j + 1)],
                                  xin_tiles[kx], kx == 0 and j % 4 == 0,
                                  kx == nkx - 1 and skip_rec))
            # Per-slot gating: slot 0 (chunks 0,1) is the locally-produced
            # self slice (Tile tracks the RAW) - ungated; slot s>=1 (chunks
            # 2s,2s+1) gates on ITS peer's semaphore, so the PE starts on
            # each slot as it arrives instead of waiting for the last one.
            last_gate = [None]

            def emit_slotted(w, tiles, nk2, wval, stopf, startf):
                for s in range(nk2 // 2):
                    args = [(out_ap(j), w[:, k, 128 * j : 128 * (j + 1)],
                             tiles[k], startf(j, k), stopf(k))
                            for j in range(8) for k in (2 * s, 2 * s + 1)]
                    wait = (rsems[s - 1], wval) if s >= 1 else None
                    g = gated_mms(args, wait, chain_to=(
                        last_gate[0] if wait is None and s > 0 else None))
                    if wait is not None:
                        last_gate[0] = g

            if x_wait is None:   # layer 0: x from DRAM tile, ungated
                xargs = []
                for j in range(8):
                    for kx in range(nkx):
                        xargs.append((out_ap(j),
                                      wx[:, kx, 128 * j : 128 * (j + 1)],
                                      xin_tiles[kx], kx == 0 and j % 4 == 0,
                                      kx == nkx - 1 and skip_rec))
                gated_mms(xargs, None)
            else:                # layer 1: x tiles are gather slots
                emit_slotted(wx, xin_tiles, nkx, x_wait[1],
                             lambda k: k == nkx - 1 and skip_rec,
                             lambda j, k: k == 0 and j % 4 == 0)
            if not skip_rec:
                wval = x_wait[1] if x_wait is not None else h_wait[1]
                if x_wait is not None:
                    # slots already gated by the x phase in FIFO order; chain
                    # peer-slot mms to the latest gate for the scheduler.
                    for s in range(4):
                        args = [(out_ap(j), wh[:, k, 128 * j : 128 * (j + 1)],
                                 h_tiles[k], False, k == 7)
                                for j in range(8) for k in (2 * s, 2 * s + 1)]
                        gated_mms(args, None, chain_to=last_gate[0])
                else:
                    emit_slotted(wh, h_tiles, 8, wval,
                                 lambda k: k == 7, lambda j, k: False)
            return pss

        def elem_quartet(ps, half, sB, c, out_bf):
            nc.scalar.activation(ps[:, 0:128], ps[:, 0:128], AF.Sigmoid,
                                 bias=sB[:, 4 * half + 0 : 4 * half + 1])
            nc.scalar.activation(ps[:, 128:256], ps[:, 128:256], AF.Sigmoid,
                                 bias=sB[:, 4 * half + 1 : 4 * half + 2])
            gsb = tmp_pool.tile([P, B], F32, name="gsb")
            nc.scalar.activation(gsb, ps[:, 256:384], AF.Tanh,
                                 bias=sB[:, 4 * half + 2 : 4 * half + 3])
            nc.scalar.activation(ps[:, 384:512], ps[:, 384:512], AF.Sigmoid,
                                 bias=sB[:, 4 * half + 3 : 4 * half + 4])
            t1 = tmp_pool.tile([P, B], F32, name="t1")
            nc.vector.tensor_mul(t1, ps[:, 0:128], gsb)              # i * g
            t2 = tmp_pool.tile([P, B], F32, name="t2")
            nc.vector.tensor_mul(t2, ps[:, 128:256], c[:, half])     # f * c
            nc.vector.tensor_add(c[:, half], t1, t2)
            tcn = tmp_pool.tile([P, B], F32, name="tc")
            nc.scalar.activation(tcn, c[:, half], AF.Tanh)
            nc.vector.tensor_mul(out_bf, ps[:, 384:512], tcn)        # o*tanh(c)

        def send_group(slot):
            """One combined send of gath[:, slot, 0:4] (h0|h1 self slices,
            1KB/partition) to the 3 XOR peers' slot-k regions. Calls on one
            SWDGE queue drain serially (~4.3us each), so split them across
            both queues (2+1) to overlap the drains."""
            src = gath[:, slot, 0:4]
            preps = {0: [], 1: []}
            for k in range(1, 4):
                rd = [None] * 8
                rd[k] = (0, k)
                q = 0 if k < 3 else 1
                prep = nc.gpsimd.remote_dma_broadcast(
                    gath[:, slot, 4 * k : 4 * (k + 1)], src,
                    rsems[k - 1], lsem, rdests=rd, queue_num=q)
                preps[q].append(prep)
                if first_prep[0] is None:
                    first_prep[0] = prep
                    add_dep_helper(prep.ins, barrier_nop.ins, sync=False,
                                   reason="sends after entry barrier")
            for q in (0, 1):
                trig = nc.gpsimd.trigger_dma(count=None, queue_num=q)
                for prep in preps[q]:
                    add_dep_helper(trig.ins, prep.ins, sync=False,
                                   reason="trigger after its queue's preps")

        # ---------------- main loop ----------------
        xt4 = None
        for t in range(n_steps):
            if t % 4 == 0:
                xt4 = xt_pool.tile([P, 4, 4, B], BF16, name="xt")
                nc.sync.dma_start(
                    out=xt4,
                    in_=xt_in[t // 4].rearrange("p (s k b) -> p s k b", s=4, k=4))
            gslot = gath[:, t % 3]
            gprev = gath[:, (t - 1) % 3]

            # layer 0, time t
            pss0 = layer_mms(
                sWx0, 4, [xt4[:, t % 4, kx] for kx in range(4)], None,
                sWh0, [h0c(gprev, j) for j in range(8)],
                (None, 2 * t), t == 0)
            for half in range(2):
                elem_quartet(pss0[half], half, sB0, cst[0], gslot[:, half])

            # layer 1, time t-1
            if t >= 1:
                pss1 = layer_mms(
                    sWx1, 8, [h0c(gprev, j) for j in range(8)], (True, 2 * t),
                    sWh1, [h1c(gprev, j) for j in range(8)],
                    None, t == 1)
                for half in range(2):
                    elem_quartet(pss1[half], half, sB1, cst[1],
                                 gslot[:, 2 + half])

            send_group(t % 3)

        # tail: layer 1, time n_steps-1 (program step tn)
        tn = n_steps
        gprev = gath[:, (tn - 1) % 3]
        gslot = gath[:, tn % 3]
        pss1 = layer_mms(
            sWx1, 8, [h0c(gprev, j) for j in range(8)], (True, 2 * tn),
            sWh1, [h1c(gprev, j) for j in range(8)], None, False)
        for half in range(2):
            elem_quartet(pss1[half], half, sB1, cst[1], gslot[:, 2 + half])
        send_group(tn % 3)

        # ---------------- head: y^T = ELU(Wbr @ h1_last + bbr) -------------
        gl = gath[:, tn % 3]
        psh = ps_pool.tile([P, 512], F32, name="ps")
        for s in range(4):
            hargs = [(psh[:, 128 * jo : 128 * (jo + 1)],
                      sWbr[:, k, 128 * jo : 128 * (jo + 1)],
                      h1c(gl, k), k == 0 and jo == 0, k == 7)
                     for jo in range(2) for k in (2 * s, 2 * s + 1)]
            gated_mms(hargs, (rsems[s - 1], 2 * (tn + 1)) if s >= 1 else None)
        for jo in range(2):
            pc = psh[:, 128 * jo : 128 * (jo + 1)]
            xv = hd_pool.tile([P, B], F32, name="xv")
            nc.scalar.activation(xv, pc, AF.Identity, bias=sBbr[:, jo : jo + 1])
            rl = hd_pool.tile([P, B], F32, name="rl")
            nc.vector.tensor_scalar_max(rl, xv, 0.0)
            mn = hd_pool.tile([P, B], F32, name="mn")
            nc.vector.tensor_scalar_min(mn, xv, 0.0)
            ex = hd_pool.tile([P, B], F32, name="ex")
            nc.scalar.activation(ex, mn, AF.Exp)
            s1 = hd_pool.tile([P, B], F32, name="s1")
            nc.vector.tensor_add(s1, rl, ex)
            yv = hd_pool.tile([P, B], F32, name="yv")
            nc.vector.tensor_scalar_add(yv, s1, -1.0)
            nc.sync.dma_start(out=y_out[jo], in_=yv)
        stack.close()

    nc._bir_kernel_barrier_sem_replica_groups.append(set(range(NUM_CORES)))
    barrier_nop.wait_op(nc._bir_kernel_barrier_sem, nc.bir_kernel_barrier_sem_inc,
                        "sem-ge", check=False)
    for inst, sem, val in patches:
        if val > 0:
            inst.wait_op(sem, val, "sem-ge", check=False)
    return patches


def build_program(n_steps=T):
    nc = bacc.Bacc("TRN2", target_bir_lowering=False, debug=False,
                   num_devices=NUM_CORES, num_swdge_queues=2)
    _build(nc, n_steps)
    nc.compile()
    return nc


def prepare_inputs(X, W_ih0, W_hh0, b_ih0, b_hh0, W_ih1, W_hh1, b_ih1, b_hh1,
                   W_br, b_br, n_steps=T):
    X = np.asarray(X, np.float32)
    bf = ml_dtypes.bfloat16
    # X^T packed 4 steps per row-block: [T/4, p, (step, k, b)]
    XT = (X[:, :n_steps].transpose(1, 2, 0)         # [T, D, B]
          .reshape(n_steps // 4, 4, 4, P, B)        # [T4, s, k, p, b]
          .transpose(0, 3, 1, 2, 4)                 # [T4, p, s, k, b]
          .reshape(n_steps // 4, P, 2048))
    XT = np.ascontiguousarray(XT).astype(bf)
    maps4 = []
    for s in range(4):
        cols = np.concatenate(
            [g * H + np.arange(HL * s + P * h, HL * s + P * h + P)
             for h in range(2) for g in range(4)])
        perm = np.concatenate(
            [np.arange(HL * (s ^ k), HL * (s ^ k) + HL) for k in range(4)])

        def w(a):
            return np.ascontiguousarray(np.asarray(a, np.float32)).astype(bf)

        b0 = np.asarray(b_ih0 + b_hh0, np.float32)[cols]
        b1 = np.asarray(b_ih1 + b_hh1, np.float32)[cols]
        maps4.append({
            "XT": XT,
            "Wx0": w(np.asarray(W_ih0).T[:, cols]),
            "Wh0": w(np.asarray(W_hh0).T[perm][:, cols]),
            "Wx1": w(np.asarray(W_ih1).T[perm][:, cols]),
            "Wh1": w(np.asarray(W_hh1).T[perm][:, cols]),
            "Wbr": w(np.asarray(W_br).T[perm]),
            "b0p": np.ascontiguousarray(b0.reshape(8, P).T),
            "b1p": np.ascontiguousarray(b1.reshape(8, P).T),
            "bbrp": np.ascontiguousarray(
                np.asarray(b_br, np.float32).reshape(2, P).T),
        })
    return [maps4[r % 4] for r in range(NUM_CORES)]


def collect(results):
    return np.ascontiguousarray(
        results[0]["y"].reshape(BR, B).T).astype(np.float32)


_cached_nc = None


def kernel(**inputs):
    global _cached_nc
    if _cached_nc is None:
        _cached_nc = build_program(T)
    in_maps = prepare_inputs(**inputs, n_steps=T)
    res = run_bass_kernel_spmd(_cached_nc, in_maps, list(range(NUM_CORES)))
    return collect(res.results)

